# revision 36
# baseline (speedup 1.0000x reference)
"""PointNet++ FeaturePropagation Trainium2 kernel (8-core SPMD).

Per core c of 8: batch b = c//2, query-half h = c%2 (8192 original points).
  1. PE computes KNN scores s(q,j) = 2 q.xyz_j - ||xyz_j||^2 against all
     4096 sampled points (top-3 score == 3 nearest neighbors, exact).
  2. DVE max/max_index extract top-3 values + indices; inverse-distance
     weights computed on-chip.
  3. Indirect DMA gathers sampled_features rows by on-chip indices.
  4. Weighted interp -> PE transpose to [C, q]; two pointwise MLP layers in
     channel-major layout; BatchNorm stats via 8-core AllReduce (conv biases
     cancel through BN and are skipped); ReLU fused into BN apply on ACT.
Host does layout transforms (transpose/reshape) and the final unshard.
"""

import numpy as np

import concourse.bass as bass
import concourse.bacc as bacc
import concourse.mybir as mybir
import concourse.tile as tile
from concourse.bass import IndirectOffsetOnAxis

F32 = mybir.dt.float32
U32 = mybir.dt.uint32
ALU = mybir.AluOpType
ACT = mybir.ActivationFunctionType
AX = mybir.AxisListType

B, S, N = 4, 4096, 16384
CS, CO = 256, 128
C1, C2 = 256, 256
NCORES = 8
QP = N // 2
BN_EPS = 1e-5
W_EPS = 1e-8
BISECT_NO_GATHER = False
COPY_DVE = ()
COPY_DMA = ()
NBATCH = 3


def build_program(n_t=QP // 128, gs=2, n_cores=NCORES, n_points_total=None):
    nq = n_t * 128
    nst = S // 128
    if n_points_total is None:
        n_points_total = n_cores * nq
    nc = bacc.Bacc("TRN2", target_bir_lowering=False, debug=False,
                   num_devices=n_cores)

    d_xT4 = nc.dram_tensor("xT4", [32, nq], F32, kind="ExternalInput")
    d_onat = nc.dram_tensor("onat", [128, n_t, 3], F32, kind="ExternalInput")
    d_snat = nc.dram_tensor("snat", [128, nst, 3], F32, kind="ExternalInput")
    d_sT3 = nc.dram_tensor("sT3", [3, S], F32, kind="ExternalInput")
    d_sfeat = nc.dram_tensor("sfeat", [S, CS], F32, kind="ExternalInput")
    d_ofT = nc.dram_tensor("ofT", [CO, nq], F32, kind="ExternalInput")
    d_w0T = nc.dram_tensor("w0T", [128, 3, C1], F32, kind="ExternalInput")
    d_w1T = nc.dram_tensor("w1T", [128, 2, C2], F32, kind="ExternalInput")
    d_g0 = nc.dram_tensor("g0p", [128, 2], F32, kind="ExternalInput")
    d_bt0 = nc.dram_tensor("bt0p", [128, 2], F32, kind="ExternalInput")
    d_g1 = nc.dram_tensor("g1p", [128, 2], F32, kind="ExternalInput")
    d_bt1 = nc.dram_tensor("bt1p", [128, 2], F32, kind="ExternalInput")
    d_eye = nc.dram_tensor("eye", [128, 128], F32, kind="ExternalInput")
    d_out = nc.dram_tensor("yT", [2, 128, nq], F32, kind="ExternalOutput")

    with tile.TileContext(nc) as tc:
        with (
            tc.tile_pool(name="const", bufs=1) as cpool,
            tc.tile_pool(name="big", bufs=1) as bigp,
            tc.tile_pool(name="sc_sb", bufs=3) as scp,
            tc.tile_pool(name="small", bufs=3) as smp,
            tc.tile_pool(name="gath", bufs=3) as gathp,
            tc.tile_pool(name="ps_sc", bufs=4, space="PSUM") as ps_sc,
            tc.tile_pool(name="ps_sm", bufs=2, space="PSUM") as ps_sm,
            tc.tile_pool(name="ps_mm1", bufs=2, space="PSUM") as ps_mm1,
            tc.tile_pool(name="dram", bufs=1, space="DRAM") as dramp,
        ):
            # ---------------- resident loads ----------------
            def load(pool, name, dram, shape):
                t_ = pool.tile(shape, F32, tag=name)
                nc.sync.dma_start(t_[:], dram[:])
                return t_

            eye = load(cpool, "eye", d_eye, [128, 128])
            w0T = load(cpool, "w0T", d_w0T, [128, 3, C1])
            w1T = load(cpool, "w1T", d_w1T, [128, 2, C2])
            g0p = load(cpool, "g0p", d_g0, [128, 2])
            bt0p = load(cpool, "bt0p", d_bt0, [128, 2])
            g1p = load(cpool, "g1p", d_g1, [128, 2])
            bt1p = load(cpool, "bt1p", d_bt1, [128, 2])

            # ---------------- norms ----------------
            onat = scp.tile([128, n_t, 3], F32, tag="scores")
            nc.sync.dma_start(onat[:], d_onat[:])
            osq = scp.tile([128, n_t, 3], F32, tag="scores")
            nc.vector.tensor_tensor(out=osq[:], in0=onat[:], in1=onat[:],
                                    op=ALU.mult)
            xn2 = cpool.tile([128, n_t], F32, tag="xn2")
            nc.vector.tensor_reduce(out=xn2[:], in_=osq[:], axis=AX.X,
                                    op=ALU.add)
            snat = scp.tile([128, nst, 3], F32, tag="scores")
            nc.sync.dma_start(snat[:], d_snat[:])
            ssq = scp.tile([128, nst, 3], F32, tag="scores")
            nc.vector.tensor_tensor(out=ssq[:], in0=snat[:], in1=snat[:],
                                    op=ALU.mult)
            sn2 = cpool.tile([128, nst], F32, tag="sn2")
            nc.vector.tensor_reduce(out=sn2[:], in_=ssq[:], axis=AX.X,
                                    op=ALU.add)
            sn2n = cpool.tile([128, nst], F32, tag="sn2n")
            nc.vector.tensor_scalar_mul(sn2n[:], sn2[:], -1.0)
            ps_t = ps_sm.tile([nst, 128], F32, tag="ps_small")
            nc.tensor.transpose(ps_t[:], sn2n[:], eye[:])
            sn2T = cpool.tile([nst, 128], F32, tag="sn2T")
            nc.scalar.copy(sn2T[:], ps_t[:])
            sT3 = scp.tile([3, S], F32, tag="scores")
            nc.sync.dma_start(sT3[:], d_sT3[:])
            rhsS = cpool.tile([32, S], F32, tag="rhsS")
            nc.vector.memset(rhsS[:], 0.0)
            nc.vector.tensor_scalar_mul(rhsS[0:3, :], sT3[:], 2.0)
            nc.gpsimd.dma_start(
                rhsS[3:4, :].rearrange("p (a b) -> p a b", a=nst), sn2T[:])

            y0a = bigp.tile([128, nq], F32, tag="y0a")
            y0b = bigp.tile([128, nq], F32, tag="y0b")
            idx_all = bigp.tile([128, n_t, 3], U32, tag="idx_all")
            w_all = bigp.tile([128, n_t, 3], F32, tag="w_all")

            # ---------------- gather + interp + matmul0 ----------------
            # build 16-partition-wrapped int16 index tensor for dma_gather:
            # flat order i = t*384 + k*128 + q -> [i%16, i//16]; since
            # 384%16==128%16==0, partition = q%16 and the col layout is
            # [t*24 + k*8 + q//16] -> 8 strided DMAs (one per q//16 block).
            idx16 = bigp.tile([128, n_t, 3], mybir.dt.int16, tag="idx16")
            ncols = n_t * 24
            wrapped = bigp.tile([128, ncols], mybir.dt.int16, tag="wrapped")
            n_g = (n_t + gs - 1) // gs

            def build_wrapped(ta, tb):
                c0, c1 = ta * 24, tb * 24
                nc.vector.tensor_copy(idx16[:, ta:tb, :],
                                      idx_all[:, ta:tb, :])
                for u in range(8):
                    nc.gpsimd.dma_start(
                        wrapped[0:16, c0 + u:c1:8].rearrange(
                            "p (t k) -> p t k", k=3),
                        idx16[16 * u:16 * (u + 1), ta:tb, :])
                for rep in range(1, 8):
                    nc.gpsimd.dma_start(
                        wrapped[16 * rep:16 * (rep + 1), c0:c1],
                        wrapped[0:16, c0:c1])

            def gather_group(g):
                t0 = g * gs
                gt = min(gs, n_t - t0)
                gbuf = gathp.tile([128, gs * 3, CS], F32, tag="gath")
                ofTg = smp.tile([CO, gs * 128], F32, tag="ofTg")
                nc.sync.dma_start(ofTg[:, 0:gt * 128],
                                  d_ofT[:, t0 * 128:(t0 + gt) * 128])
                nc.gpsimd.dma_gather(
                    out_ap=gbuf[:, 0:gt * 3, :],
                    in_ap=d_sfeat[:],
                    idxs_ap=wrapped[:, t0 * 24:(t0 + gt) * 24],
                    num_idxs=gt * 384,
                    num_idxs_reg=gt * 384,
                    elem_size=CS,
                )
                for tt in range(gt):
                    t = t0 + tt
                    interp = smp.tile([128, CS], F32, tag="interp", bufs=2)
                    acc = smp.tile([128, CS], F32, tag="interp_acc", bufs=2)
                    nc.vector.tensor_scalar(
                        out=acc[:], in0=gbuf[:, tt * 3, :],
                        scalar1=w_all[:, t, 0:1], scalar2=None, op0=ALU.mult)
                    nc.vector.affine_then_add(
                        out=interp[:], in0=gbuf[:, tt * 3 + 1, :],
                        in1=acc[:], scale=w_all[:, t, 1:2], bias=0.0)
                    nc.vector.affine_then_add(
                        out=acc[:], in0=gbuf[:, tt * 3 + 2, :],
                        in1=interp[:], scale=w_all[:, t, 2:3], bias=0.0)
                    interp = acc
                    iT = smp.tile([128, 2, 128], F32, tag="interpT")
                    for hh in range(2):
                        ps_tr = ps_sm.tile([128, 128], F32, tag="ps_small")
                        nc.tensor.transpose(
                            ps_tr[:], interp[:, hh * 128:(hh + 1) * 128],
                            eye[:])
                        nc.vector.tensor_copy(iT[:, hh, :], ps_tr[:])
                    for m, ybuf in ((0, y0a), (1, y0b)):
                        ps_y = ps_sm.tile([128, 128], F32, tag="ps_small")
                        mcol = slice(m * 128, (m + 1) * 128)
                        nc.tensor.matmul(ps_y[:], w0T[:, 0, mcol],
                                         ofTg[:, tt * 128:(tt + 1) * 128],
                                         start=True, stop=False)
                        nc.tensor.matmul(ps_y[:], w0T[:, 1, mcol],
                                         iT[:, 0, :], start=False, stop=False)
                        nc.tensor.matmul(ps_y[:], w0T[:, 2, mcol],
                                         iT[:, 1, :], start=False, stop=True)
                        nc.scalar.copy(ybuf[:, t * 128:(t + 1) * 128], ps_y[:])


            # gather-batch schedule: spread wrapped-build + gathers under
            # the KNN loop; only the last batch remains in the tail.
            nb = NBATCH if n_t >= 4 * gs * NBATCH else (2 if n_t >= 8 else 1)
            bt_sz = ((n_t // nb) // gs) * gs
            fire_at = {}
            prev = 0
            for i in range(nb):
                tb = n_t if i == nb - 1 else min((i + 1) * bt_sz, n_t)
                if tb > prev:
                    fire_at[tb - 1] = (prev, tb)
                    prev = tb

            # ---------------- KNN selection ----------------
            xg = None
            for t in range(n_t):
                if t % 8 == 0:
                    xg = smp.tile([32, 8 * 128], F32, tag="xT4g")
                    gcols = min(8 * 128, nq - t * 128)
                    nc.sync.dma_start(xg[:, 0:gcols],
                                      d_xT4[:, t * 128:t * 128 + gcols])
                tl = (t % 8) * 128
                sc = scp.tile([128, S], F32, tag="scores")
                for q8 in range(8):
                    ps = ps_sc.tile([128, 512], F32, tag="ps_score")
                    col = q8 * 512
                    nc.tensor.matmul(ps[:], xg[:, tl:tl + 128],
                                     rhsS[:, col:col + 512],
                                     start=True, stop=True)
                    # spread PSUM->SBUF copies across engines: ACT binds the
                    # loop, DVE and the DMA queue have slack
                    if q8 in COPY_DVE:
                        nc.vector.tensor_copy(sc[:, col:col + 512], ps[:])
                    elif q8 in COPY_DMA:
                        nc.sync.dma_start(sc[:, col:col + 512], ps[:])
                    else:
                        nc.scalar.copy(sc[:, col:col + 512], ps[:])

                v8 = smp.tile([128, 8], F32, tag="v8")
                nc.vector.max(v8[:], sc[:])
                i8 = smp.tile([128, 8], U32, tag="i8")
                nc.vector.max_index(i8[:], v8[:], sc[:])
                nc.vector.tensor_copy(idx_all[:, t, :], i8[:, 0:3])

                d3 = smp.tile([128, 3], F32, tag="d3")
                nc.vector.tensor_scalar(
                    out=d3[:], in0=v8[:, 0:3],
                    scalar1=xn2[:, t:t + 1], scalar2=-1.0,
                    op0=ALU.subtract, op1=ALU.mult)
                nc.vector.tensor_scalar_add(d3[:], d3[:], W_EPS)
                r3 = smp.tile([128, 3], F32, tag="r3")
                nc.vector.reciprocal(r3[:], d3[:])
                rs = smp.tile([128, 1], F32, tag="rs")
                nc.vector.tensor_reduce(out=rs[:], in_=r3[:], axis=AX.X,
                                        op=ALU.add)
                rsr = smp.tile([128, 1], F32, tag="rsr")
                nc.vector.reciprocal(rsr[:], rs[:])
                nc.vector.tensor_scalar(
                    out=w_all[:, t, :], in0=r3[:], scalar1=rsr[:],
                    scalar2=None, op0=ALU.mult)

                if t in fire_at:
                    ta, tb = fire_at[t]
                    build_wrapped(ta, tb)
                    for g_ in range(ta // gs, (tb + gs - 1) // gs):
                        gather_group(g_)

            # ---------------- BN helpers ----------------
            def bn_allreduce(ya, yb, gp, btp, tag, pre=None):
                """Returns (a, bhat) [128,2] with yhat = Relu(y*a + bhat).
                pre=(sump, sqp, nchunk): partial sums already accumulated."""
                if pre is None:
                    csz = min(1024, nq)
                    nchunk = nq // csz
                    sump = smp.tile([128, 2 * nchunk], F32, tag=f"sump{tag}")
                    sqp = smp.tile([128, 2 * nchunk], F32, tag=f"sqp{tag}")
                    scratch = cpool.tile([128, csz], F32, tag="bn_scratch")
                    for m, ybuf in ((0, ya), (1, yb)):
                        for ch in range(nchunk):
                            sl = slice(ch * csz, (ch + 1) * csz)
                            col = m * nchunk + ch
                            nc.vector.tensor_reduce(
                                out=sump[:, col:col + 1], in_=ybuf[:, sl],
                                axis=AX.X, op=ALU.add)
                            nc.scalar.activation(
                                out=scratch[:], in_=ybuf[:, sl],
                                func=ACT.Square,
                                accum_out=sqp[:, col:col + 1])
                else:
                    sump, sqp, nchunk = pre
                stats = smp.tile([128, 4], F32, tag=f"stats{tag}")
                nc.vector.tensor_reduce(
                    out=stats[:, 0:2],
                    in_=sump[:].rearrange("p (m c) -> p m c", m=2),
                    axis=AX.X, op=ALU.add)
                nc.vector.tensor_reduce(
                    out=stats[:, 2:4],
                    in_=sqp[:].rearrange("p (m c) -> p m c", m=2),
                    axis=AX.X, op=ALU.add)
                bi = dramp.tile([128, 4], F32, tag=f"bi{tag}")
                bo = dramp.tile([128, 4], F32, tag=f"bo{tag}")
                nc.gpsimd.dma_start(bi[:], stats[:])
                nc.gpsimd.collective_compute(
                    "AllReduce", ALU.add,
                    replica_groups=[list(range(n_cores))],
                    ins=[bi.opt()], outs=[bo.opt()])
                gstats = smp.tile([128, 4], F32, tag=f"gstats{tag}")
                nc.gpsimd.dma_start(gstats[:], bo[:])

                mean = smp.tile([128, 2], F32, tag=f"mean{tag}")
                nc.vector.tensor_scalar_mul(mean[:], gstats[:, 0:2],
                                            1.0 / n_points_total)
                vpe = smp.tile([128, 2], F32, tag=f"vpe{tag}")
                nc.vector.tensor_scalar_mul(vpe[:], gstats[:, 2:4],
                                            1.0 / n_points_total)
                msq = smp.tile([128, 2], F32, tag=f"msq{tag}")
                nc.vector.tensor_tensor(out=msq[:], in0=mean[:], in1=mean[:],
                                        op=ALU.mult)
                nc.vector.tensor_tensor(out=vpe[:], in0=vpe[:], in1=msq[:],
                                        op=ALU.subtract)
                nc.vector.tensor_scalar_add(vpe[:], vpe[:], BN_EPS)
                rcp = smp.tile([128, 2], F32, tag=f"rcp{tag}")
                nc.vector.reciprocal(rcp[:], vpe[:])
                rsq = smp.tile([128, 2], F32, tag=f"rsq{tag}")
                nc.scalar.activation(out=rsq[:], in_=rcp[:], func=ACT.Sqrt)
                t1 = smp.tile([128, 2], F32, tag=f"t1{tag}")
                nc.vector.tensor_tensor(out=t1[:], in0=rsq[:], in1=rsq[:],
                                        op=ALU.mult)
                nc.vector.tensor_tensor(out=t1[:], in0=t1[:], in1=vpe[:],
                                        op=ALU.mult)
                nc.vector.tensor_scalar(out=t1[:], in0=t1[:], scalar1=-0.5,
                                        scalar2=1.5, op0=ALU.mult, op1=ALU.add)
                nc.vector.tensor_tensor(out=rsq[:], in0=rsq[:], in1=t1[:],
                                        op=ALU.mult)
                a = smp.tile([128, 2], F32, tag=f"a{tag}")
                nc.vector.tensor_tensor(out=a[:], in0=gp[:], in1=rsq[:],
                                        op=ALU.mult)
                bhat = smp.tile([128, 2], F32, tag=f"bhat{tag}")
                nc.vector.tensor_tensor(out=bhat[:], in0=mean[:], in1=a[:],
                                        op=ALU.mult)
                nc.vector.tensor_tensor(out=bhat[:], in0=btp[:], in1=bhat[:],
                                        op=ALU.subtract)
                return a, bhat

            # ---------------- layer 1 ----------------
            a0, b0h = bn_allreduce(y0a, y0b, g0p, bt0p, "0")
            csz = min(512, nq)
            for ch in range(nq // csz):
                sl = slice(ch * csz, (ch + 1) * csz)
                yh0 = smp.tile([128, csz], F32, tag="yh0")
                yh1 = smp.tile([128, csz], F32, tag="yh1")
                nc.scalar.activation(out=yh0[:], in_=y0a[:, sl], func=ACT.Relu,
                                     scale=a0[:, 0:1], bias=b0h[:, 0:1])
                nc.scalar.activation(out=yh1[:], in_=y0b[:, sl], func=ACT.Relu,
                                     scale=a0[:, 1:2], bias=b0h[:, 1:2])
                for m, ybuf in ((0, y0a), (1, y0b)):
                    ps1 = ps_mm1.tile([128, csz], F32, tag="ps_mm1")
                    mcol = slice(m * 128, (m + 1) * 128)
                    nc.tensor.matmul(ps1[:], w1T[:, 0, mcol], yh0[:],
                                     start=True, stop=False)
                    nc.tensor.matmul(ps1[:], w1T[:, 1, mcol], yh1[:],
                                     start=False, stop=True)
                    nc.scalar.copy(ybuf[:, sl], ps1[:])

            # ---------------- layer 2 BN + out ----------------
            a1, b1h = bn_allreduce(y0a, y0b, g1p, bt1p, "1")
            csz2 = min(1024, nq)
            for ch in range(nq // csz2):
                sl = slice(ch * csz2, (ch + 1) * csz2)
                for m, ybuf in ((0, y0a), (1, y0b)):
                    o = smp.tile([128, csz2], F32, tag="outsb", bufs=2)
                    nc.scalar.activation(out=o[:], in_=ybuf[:, sl],
                                         func=ACT.Relu,
                                         scale=a1[:, m:m + 1],
                                         bias=b1h[:, m:m + 1])
                    nc.sync.dma_start(d_out[m, :, sl], o[:])

    nc.compile()
    return nc


def make_core_inputs(sampled_xyz, sampled_features, original_xyz,
                     original_features, w0, w1, g0, bt0, g1, bt1,
                     core, n_t=QP // 128):
    """Host-side layout prep for one core (transposes/reshapes only)."""
    nq = n_t * 128
    nst = S // 128
    b, h = core // 2, core % 2
    ox = original_xyz[b, h * nq:(h + 1) * nq]          # [nq, 3]
    of = original_features[b, h * nq:(h + 1) * nq]     # [nq, CO]
    sx = sampled_xyz[b]                                # [S, 3]
    f32 = np.float32
    xT4 = np.concatenate([ox.T, np.ones((1, nq), f32),
                          np.zeros((28, nq), f32)], 0).astype(f32)
    onat = np.ascontiguousarray(
        ox.reshape(n_t, 128, 3).transpose(1, 0, 2)).astype(f32)
    snat = np.ascontiguousarray(
        sx.reshape(nst, 128, 3).transpose(1, 0, 2)).astype(f32)
    return {
        "xT4": xT4,
        "onat": onat,
        "snat": snat,
        "sT3": np.ascontiguousarray(sx.T).astype(f32),
        "sfeat": np.ascontiguousarray(sampled_features[b]).astype(f32),
        "ofT": np.ascontiguousarray(of.T).astype(f32),
        "w0T": np.ascontiguousarray(
            w0.T.reshape(3, 128, C1).transpose(1, 0, 2)).astype(f32),
        "w1T": np.ascontiguousarray(
            w1.T.reshape(2, 128, C2).transpose(1, 0, 2)).astype(f32),
        "g0p": np.ascontiguousarray(g0.reshape(2, 128).T).astype(f32),
        "bt0p": np.ascontiguousarray(bt0.reshape(2, 128).T).astype(f32),
        "g1p": np.ascontiguousarray(g1.reshape(2, 128).T).astype(f32),
        "bt1p": np.ascontiguousarray(bt1.reshape(2, 128).T).astype(f32),
        "eye": np.eye(128, dtype=f32),
    }


_PROGRAM_CACHE = {}


def kernel(sampled_xyz, sampled_features, original_xyz, original_features,
           w0, b0, g0, bt0, w1, b1, g1, bt1, k):
    assert int(k) == 3
    from concourse.bass_utils import run_bass_kernel_spmd

    key = "full"
    if key not in _PROGRAM_CACHE:
        _PROGRAM_CACHE[key] = build_program()
    nc = _PROGRAM_CACHE[key]

    args = (sampled_xyz, sampled_features, original_xyz, original_features,
            w0, w1, g0, bt0, g1, bt1)
    in_maps = [make_core_inputs(*[np.asarray(a, np.float32) for a in args],
                                core=c) for c in range(NCORES)]
    res = run_bass_kernel_spmd(nc, in_maps, core_ids=list(range(NCORES)))
    out = np.empty((B, N, C2), np.float32)
    nq = QP
    for c in range(NCORES):
        b, h = c // 2, c % 2
        yT = res.results[c]["yT"]            # [2, 128, nq]
        y = yT.reshape(256, nq).T            # [nq, 256]
        out[b, h * nq:(h + 1) * nq] = y
    return out

